# revision 1
# baseline (speedup 1.0000x reference)
"""Causal GQA self-attention (B=2, S=2048, D=2048, 16 Q heads / 4 KV heads)
on 8 Trainium2 NeuronCores.

Sharding: core i handles (batch b = i // 4, kv-head group g = i % 4) — one
batch element and 4 consecutive query heads + their shared KV head. Each
core computes its heads' attention and a partial output projection
(columns g*512:(g+1)*512 of wo contracted); the host sums the 4 partials
per batch (fp32) and adds the output bias.

On-chip layout is fully transposed ("T" = feature-on-partition):
  xT [d, s]   QT/KT/VT [e, s]   scores S^T [k, q]   A^T = exp(S^T)
  Y^T [e, q] = sum_k V[k, e] A^T[k, q]   outT [f, s] partials (bf16)
This keeps every matmul's moving operand 512 wide and avoids transposing
the attention matrices; only V (16 128x128 tiles) is DMA-transposed.
Scale 1/sqrt(128) is folded into wq/bq on the host. Softmax runs without
max-subtraction (logits are N(0,1)-scaled; exp cannot overflow), masked
diagonal blocks multiply exp by a 0/1 triangle, and fully-masked column
subranges of diagonal blocks are skipped (causal FLOP skip at 128-column
granularity). The softmax denominator (a partition-dim sum) is computed
and broadcast in one shot by a 128x128 ones-matrix matmul into PSUM;
a full-width DVE reciprocal then scales Y^T.

All inputs are host-prearranged into their exact SBUF layouts [128, X] so
each tensor loads with a single dma_start (HWDGE descriptor processing is
serial, ~625ns per DMA); outputs stage per seq-chunk in SBUF and leave
via single gpsimd (SWDGE) DMAs, which run on the otherwise-idle Pool
engine path.
"""

import math

import numpy as np
import ml_dtypes

import concourse.bass as bass
import concourse.mybir as mybir
import concourse.tile as tile
from concourse.bass_utils import run_bass_kernel_spmd

BF16 = mybir.dt.bfloat16
F32 = mybir.dt.float32

B = 2
S = 2048
D = 2048
N_HEAD = 16
HEAD_DIM = 128
N_KV = 4
GQ = N_HEAD // N_KV          # heads per group = 4
EG = GQ * HEAD_DIM           # embed dims per group = 512
SC = 512                     # seq chunk (moving-dim width)
NSC = S // SC                # 4 seq chunks
NT = D // 128                # 16 d-tiles
NST = S // 128               # 16 seq k-tiles
ACT = mybir.ActivationFunctionType

_CACHE = {}


def _build_nc():
    nc = bass.Bass("TRN2", target_bir_lowering=False)

    # All inputs are host-prearranged to exact SBUF layout [128, X] so each
    # loads with ONE dma_start (HWDGE descriptor processing is serial ~625ns
    # per DMA) with maximal contiguous runs.
    xc = [nc.declare_dram_parameter(f"xc{c}", [128, NT * SC], BF16,
                                    isOutput=False) for c in range(NSC)]
    wqA = nc.declare_dram_parameter("wqA", [128, NT * EG // 2], BF16, isOutput=False)
    wqB = nc.declare_dram_parameter("wqB", [128, NT * EG // 2], BF16, isOutput=False)
    wk = nc.declare_dram_parameter("wk", [128, NT * HEAD_DIM], BF16, isOutput=False)
    wv = nc.declare_dram_parameter("wv", [128, NT * HEAD_DIM], BF16, isOutput=False)
    wo = nc.declare_dram_parameter("wo", [128, GQ * D], BF16, isOutput=False)
    biases = nc.declare_dram_parameter("biases", [128, 6], F32, isOutput=False)
    masks = nc.declare_dram_parameter("masks", [128, 128], BF16, isOutput=False)
    outT = nc.declare_dram_parameter("outT", [D, S], BF16, isOutput=True)

    with tile.TileContext(nc) as tc:
        with (
            tc.tile_pool(name="persist", bufs=1) as pp,
            tc.tile_pool(name="rot", bufs=1) as rp,
            tc.tile_pool(name="ps", bufs=1, space="PSUM") as ps,
        ):
            # ---- constants (no DMA deps) ----
            ones_sq = pp.tile([128, 128], BF16, name="ones_sq")
            nc.vector.memset(ones_sq[:], 1.0)

            # ---- batched loads, priority order (HWDGE is serial) ----
            wk_sb = pp.tile([128, NT * HEAD_DIM], BF16, name="wk_sb")
            nc.sync.dma_start(wk_sb[:], wk[:])
            x_sb = [None] * NSC
            x0_parts = []
            for q in range(4):
                t = pp.tile([128, NT * SC // 4], BF16, name=f"x_sb0{q}")
                nc.sync.dma_start(
                    t[:], xc[0][:, q * NT * SC // 4:(q + 1) * NT * SC // 4])
                x0_parts.append(t)
            wv_sb = pp.tile([128, NT * HEAD_DIM], BF16, name="wv_sb")
            nc.sync.dma_start(wv_sb[:], wv[:])
            wqA_sb = pp.tile([128, NT * EG // 2], BF16, name="wqA_sb")
            nc.sync.dma_start(wqA_sb[:], wqA[:])
            wqB_sb = pp.tile([128, NT * EG // 2], BF16, name="wqB_sb")
            nc.sync.dma_start(wqB_sb[:], wqB[:])
            b_sb = pp.tile([128, 6], F32, name="b_sb")
            nc.sync.dma_start(b_sb[:], biases[:])
            m_sb = pp.tile([128, 128], BF16, name="m_sb")
            nc.sync.dma_start(m_sb[:], masks[:])
            for c in range(1, NSC):
                x_sb[c] = pp.tile([128, NT * SC], BF16, name=f"x_sb{c}")
                nc.sync.dma_start(x_sb[c][:], xc[c][:])
            wo_sb = pp.tile([128, GQ * D], BF16, name="wo_sb")
            nc.sync.dma_start(wo_sb[:], wo[:])

            def x_t(dt_i, c):
                if c == 0:
                    t = x0_parts[dt_i // 4]
                    j = dt_i % 4
                    return t[:, j * SC:(j + 1) * SC]
                return x_sb[c][:, dt_i * SC:(dt_i + 1) * SC]

            def wk_t(dt_i):
                return wk_sb[:, dt_i * HEAD_DIM:(dt_i + 1) * HEAD_DIM]

            def wv_t(dt_i):
                return wv_sb[:, dt_i * HEAD_DIM:(dt_i + 1) * HEAD_DIM]

            def wq_t(dt_i, h):
                t = wqA_sb if h < 2 else wqB_sb
                hh = h % 2
                half = EG // 2
                return t[:, dt_i * half + hh * 128:dt_i * half + (hh + 1) * 128]

            def wo_t(et, ft):
                return wo_sb[:, et * D + ft * 128:et * D + (ft + 1) * 128]

            KT_sb, VT_sb, V_sb = [None] * NSC, [None] * NSC, [None] * NST
            QT = [[None] * NSC for _ in range(GQ)]
            Ynorm = [[None] * NSC for _ in range(GQ)]

            def projections(c):
                # K/V projections for chunk c
                for w_t, bias_ap, out_list, label in (
                    (wk_t, b_sb[:, 4:5], KT_sb, "KT"),
                    (wv_t, b_sb[:, 5:6], VT_sb, "VT"),
                ):
                    psum = ps.tile([128, SC], F32, name=f"{label}ps{c}",
                                   tag="mm", bufs=5)
                    for dt_i in range(NT):
                        nc.tensor.matmul(
                            psum[:], w_t(dt_i), x_t(dt_i, c),
                            start=(dt_i == 0), stop=(dt_i == NT - 1))
                    o = pp.tile([128, SC], BF16, name=f"{label}{c}")
                    nc.scalar.activation(o[:], psum[:], ACT.Identity,
                                         bias=bias_ap)
                    out_list[c] = o

                # V natural tiles for this chunk (DMA transpose, off PE)
                for j in range(4):
                    st = 4 * c + j
                    v = pp.tile([128, 128], BF16, name=f"V{st}")
                    nc.sync.dma_start(
                        v[:], VT_sb[c][:, j * 128:(j + 1) * 128],
                        transpose=True)
                    V_sb[st] = v

                # Q projections for all heads, chunk c
                for h in range(GQ):
                    psum = ps.tile([128, SC], F32, name=f"QTps{h}_{c}",
                                   tag="mm", bufs=5)
                    for dt_i in range(NT):
                        nc.tensor.matmul(
                            psum[:],
                            wq_t(dt_i, h), x_t(dt_i, c),
                            start=(dt_i == 0), stop=(dt_i == NT - 1))
                    q = pp.tile([128, SC], BF16, name=f"QT{h}_{c}")
                    nc.scalar.activation(q[:], psum[:], ACT.Identity,
                                         bias=b_sb[:, h:h + 1])
                    QT[h][c] = q

            def attention(c, interleave=False):
                n_kt = 4 * (c + 1)
                y_ps_l, sumacc_l = {}, {}

                def attn_step(h, kt, first=None, last=None):
                    if first is None:
                        first = (kt == 0)
                    if last is None:
                        last = (kt == n_kt - 1)
                    if first:
                        y_ps_l[h] = ps.tile([128, SC], F32, name=f"yps{h}_{c}",
                                            tag="y", bufs=2)
                        sumacc_l[h] = rp.tile([128, SC], F32, name=f"sum{h}_{c}",
                                              tag="sumacc", bufs=3)
                    y_ps, sumacc = y_ps_l[h], sumacc_l[h]
                    # diagonal blocks: columns j < 128r are fully masked —
                    # compute only the live subrange [j0:].
                    r = kt - 4 * c
                    j0 = 128 * r if r > 0 else 0
                    s_ps = ps.tile([128, SC], F32, name=f"sps{h}_{c}_{kt}",
                                   tag="mm", bufs=5)
                    nc.tensor.matmul(
                        s_ps[:, j0:],
                        KT_sb[kt // 4][:, (kt % 4) * 128:(kt % 4 + 1) * 128],
                        QT[h][c][:, j0:], start=True, stop=True)
                    a = rp.tile([128, SC], BF16, name=f"A{h}_{c}_{kt}",
                                tag="A", bufs=28)
                    nc.scalar.activation(a[:, j0:], s_ps[:, j0:], ACT.Exp)
                    if r >= 0:
                        # only the 128-wide diagonal block is partially
                        # masked; columns beyond it are fully unmasked
                        nc.vector.tensor_mul(a[:, j0:j0 + 128],
                                             a[:, j0:j0 + 128], m_sb[:])
                    nc.tensor.matmul(y_ps[:, j0:], V_sb[kt][:], a[:, j0:],
                                     start=first, stop=last)
                    if first:
                        nc.vector.tensor_copy(sumacc[:], a[:])
                    else:
                        nc.vector.tensor_add(sumacc[:, j0:], sumacc[:, j0:],
                                             a[:, j0:])

                def attn_tail(h):
                    y_ps, sumacc = y_ps_l[h], sumacc_l[h]
                    sum_bf = rp.tile([128, SC], BF16, name=f"sumbf{h}_{c}",
                                     tag="sumbf", bufs=3)
                    nc.vector.tensor_copy(sum_bf[:], sumacc[:])
                    # ones-matrix matmul = partition sum broadcast to all 128
                    # rows in one shot; reciprocal then runs on all lanes.
                    sum_ps = ps.tile([128, SC], F32, name=f"sumps{h}_{c}",
                                     tag="small", bufs=1)
                    nc.tensor.matmul(sum_ps[:], ones_sq[:], sum_bf[:],
                                     start=True, stop=True)
                    rb_sb = rp.tile([128, SC], F32, name=f"rb{h}_{c}",
                                    bufs=3, tag="rb")
                    nc.vector.reciprocal(rb_sb[:], sum_ps[:])
                    yn = pp.tile([128, SC], BF16, name=f"Yn{h}_{c}")
                    nc.vector.tensor_mul(yn[:], y_ps[:], rb_sb[:])
                    Ynorm[h][c] = yn

                if interleave:
                    for hp in range(0, GQ, 2):
                        for kt in range(n_kt):
                            attn_step(hp, kt)
                            attn_step(hp + 1, kt)
                        attn_tail(hp)
                        attn_tail(hp + 1)
                    return
                for h in range(GQ):
                    for kt in range(n_kt):
                        attn_step(h, kt)
                    attn_tail(h)

            def outproj(c):
                o_stage = rp.tile([128, NT * SC], BF16, name=f"ostage{c}",
                                  tag="ostage", bufs=1)
                for ft in range(NT):
                    o_ps = ps.tile([128, SC], F32, name=f"ops{c}_{ft}",
                                   tag="mm", bufs=5)
                    for et in range(GQ):
                        nc.tensor.matmul(
                            o_ps[:],
                            wo_t(et, ft), Ynorm[et][c][:],
                            start=(et == 0), stop=(et == GQ - 1))
                    dst = o_stage[:, ft * SC:(ft + 1) * SC]
                    if ft % 2 == 0:
                        nc.scalar.activation(dst, o_ps[:], ACT.Copy)
                    else:
                        nc.vector.tensor_copy(dst, o_ps[:])
                outT_v = outT.rearrange("(n p) s -> p n s", p=128)[
                    :, :, c * SC:(c + 1) * SC]
                o_stage_v = o_stage[:].rearrange("p (n j) -> p n j", j=SC)
                nsplit = 8 if c == NSC - 1 else 2
                step = NT // nsplit
                for qq in range(nsplit):
                    nc.gpsimd.dma_start(outT_v[:, qq * step:(qq + 1) * step],
                                        o_stage_v[:, qq * step:(qq + 1) * step])

            projections(0)
            for c in range(NSC):
                attention(c, interleave=(c >= 1))
                if c + 1 < NSC:
                    projections(c + 1)
                outproj(c)

    _split_multiwait(nc)
    return nc


_SPLIT_N = [0]


def _split_multiwait(nc):
    """Rewrite instructions carrying >1 semaphore wait.

    The walrus build here allows one sync wait per instruction; Tile's
    wait-assignment freely attaches several. Hoist all but the last wait
    onto fresh single-wait NoOps inserted just before the instruction in
    its basic block (engine streams are in-order, so semantics are
    unchanged).
    """
    for f in nc.m.functions:
        for bb in f.blocks:
            il = bb.instructions
            if not any(i.sync_info is not None and len(i.sync_info.on_wait) > 1
                       for i in il):
                continue
            new = []
            for inst in il:
                si = inst.sync_info
                if si is not None and len(si.on_wait) > 1:
                    waits = list(si.on_wait)
                    for w in waits[:-1]:
                        _SPLIT_N[0] += 1
                        new.append(mybir.InstNoOp(
                            name=f"I-waitsplit{_SPLIT_N[0]}",
                            engine=inst.engine,
                            bass_nofuse=True,
                            sync_info=mybir.SyncInfo(on_wait=[w], on_update=[]),
                        ))
                    inst.sync_info = mybir.SyncInfo(
                        on_wait=[waits[-1]], on_update=list(si.on_update))
                new.append(inst)
            bb.instructions = new


def _host_prep(x, wq_w, wq_b, wk_w, wk_b, wv_w, wv_b, wo_w, wo_b):
    """Build the 8 per-core input maps, prearranged to SBUF layout."""
    bf16 = ml_dtypes.bfloat16
    scale = np.float32(1.0 / math.sqrt(HEAD_DIM))

    def to_sbuf_rows(a2d, width):
        # [NT*128, width] -> [128, NT*width] with column blocks = row tiles
        nt = a2d.shape[0] // 128
        return np.ascontiguousarray(
            a2d.reshape(nt, 128, width).transpose(1, 0, 2).reshape(128, nt * width))

    xc_b = []
    for b in range(B):
        xbT = x[b].T.astype(bf16)                      # [d, s]
        arr = xbT.reshape(NT, 128, NSC, SC)            # [dt, p, c, j]
        xc_b.append([np.ascontiguousarray(
            arr[:, :, c, :].transpose(1, 0, 2).reshape(128, NT * SC))
            for c in range(NSC)])

    jj = np.arange(128, dtype=np.int32)[None, :]
    pp_ = np.arange(128, dtype=np.int32)[:, None]
    masks = (jj >= pp_).astype(bf16)

    per_g = []
    for g in range(N_KV):
        wqT = (wq_w[g * EG:(g + 1) * EG, :] * scale).T.astype(bf16)   # [d, 512]
        wkT = wk_w[g * HEAD_DIM:(g + 1) * HEAD_DIM, :].T.astype(bf16)  # [d, 128]
        wvT = wv_w[g * HEAD_DIM:(g + 1) * HEAD_DIM, :].T.astype(bf16)
        woT = wo_w[:, g * EG:(g + 1) * EG].T.astype(bf16)              # [512, f]
        biases = np.empty((128, 6), np.float32)
        biases[:, :GQ] = (wq_b[g * EG:(g + 1) * EG] * scale).reshape(GQ, 128).T
        biases[:, 4] = wk_b[g * HEAD_DIM:(g + 1) * HEAD_DIM]
        biases[:, 5] = wv_b[g * HEAD_DIM:(g + 1) * HEAD_DIM]
        per_g.append(dict(
            wqA=to_sbuf_rows(np.ascontiguousarray(wqT[:, :EG // 2]), EG // 2),
            wqB=to_sbuf_rows(np.ascontiguousarray(wqT[:, EG // 2:]), EG // 2),
            wk=to_sbuf_rows(wkT, HEAD_DIM),
            wv=to_sbuf_rows(wvT, HEAD_DIM),
            wo=to_sbuf_rows(woT, D),
            biases=biases,
        ))

    in_maps = []
    for core in range(8):
        b, g = divmod(core, N_KV)
        m = dict(per_g[g])
        for c in range(NSC):
            m[f"xc{c}"] = xc_b[b][c]
        m["masks"] = masks
        in_maps.append(m)
    return in_maps


def kernel(x, wq_w, wq_b, wk_w, wk_b, wv_w, wv_b, wo_w, wo_b, **run_kwargs):
    x = np.asarray(x, dtype=np.float32)
    wq_w = np.asarray(wq_w, dtype=np.float32)
    wq_b = np.asarray(wq_b, dtype=np.float32)
    wk_w = np.asarray(wk_w, dtype=np.float32)
    wk_b = np.asarray(wk_b, dtype=np.float32)
    wv_w = np.asarray(wv_w, dtype=np.float32)
    wv_b = np.asarray(wv_b, dtype=np.float32)
    wo_w = np.asarray(wo_w, dtype=np.float32)
    wo_b = np.asarray(wo_b, dtype=np.float32)

    if "nc" not in _CACHE:
        _CACHE["nc"] = _build_nc()
    nc = _CACHE["nc"]

    in_maps = _host_prep(x, wq_w, wq_b, wk_w, wk_b, wv_w, wv_b, wo_w, wo_b)
    res = run_bass_kernel_spmd(nc, in_maps, core_ids=list(range(8)),
                               **run_kwargs)

    out = np.empty((B, S, D), dtype=np.float32)
    for b in range(B):
        acc = res.results[b * N_KV]["outT"].astype(np.float32)
        for g in range(1, N_KV):
            acc = acc + res.results[b * N_KV + g]["outT"].astype(np.float32)
        out[b] = acc.T + wo_b[None, :]
    _CACHE["last_res"] = res
    return out



# revision 6
# speedup vs baseline: 1.0791x; 1.0791x over previous
"""Causal GQA self-attention (B=2, S=2048, D=2048, 16 Q heads / 4 KV heads)
on 8 Trainium2 NeuronCores.

Sharding: core i handles (batch b = i // 4, kv-head group g = i % 4) — one
batch element and 4 consecutive query heads + their shared KV head. Each
core computes its heads' attention and a partial output projection
(columns g*512:(g+1)*512 of wo contracted); the host sums the 4 partials
per batch (fp32) and adds the output bias.

On-chip layout is fully transposed ("T" = feature-on-partition):
  xT [d, s]   QT/KT/VT [e, s]   scores S^T [k, q]   A^T = exp(S^T)
  Y^T [e, q] = sum_k V[k, e] A^T[k, q]   outT [f, s] partials (bf16)
This keeps every matmul's moving operand 512 wide and avoids transposing
the attention matrices; only V (16 128x128 tiles) is DMA-transposed.
Scale 1/sqrt(128) is folded into wq/bq on the host. Softmax runs without
max-subtraction (logits are N(0,1)-scaled; exp cannot overflow), masked
diagonal blocks multiply exp by a 0/1 triangle, and fully-masked column
subranges of diagonal blocks are skipped (causal FLOP skip at 128-column
granularity). The softmax denominator (a partition-dim sum) is computed
and broadcast in one shot by a 128x128 ones-matrix matmul into PSUM;
a full-width DVE reciprocal then scales Y^T.

All inputs are host-prearranged into their exact SBUF layouts [128, X] so
each tensor loads with a single dma_start (HWDGE descriptor processing is
serial, ~625ns per DMA); outputs stage per seq-chunk in SBUF and leave
via single gpsimd (SWDGE) DMAs, which run on the otherwise-idle Pool
engine path.
"""

import math

import numpy as np
import ml_dtypes

import concourse.bass as bass
import concourse.mybir as mybir
import concourse.tile as tile
from concourse.bass_utils import run_bass_kernel_spmd

BF16 = mybir.dt.bfloat16
F32 = mybir.dt.float32
FP8 = mybir.dt.float8e4
DRM = mybir.MatmulPerfMode.DoubleRow
SW = 64.0          # fp8 weight upscale; compensated by Act scale 1/SW

B = 2
S = 2048
D = 2048
N_HEAD = 16
HEAD_DIM = 128
N_KV = 4
GQ = N_HEAD // N_KV          # heads per group = 4
EG = GQ * HEAD_DIM           # embed dims per group = 512
SC = 512                     # seq chunk (moving-dim width)
NSC = S // SC                # 4 seq chunks
NT = D // 128                # 16 d-tiles
NST = S // 128               # 16 seq k-tiles
ACT = mybir.ActivationFunctionType

_CACHE = {}


def _build_nc():
    nc = bass.Bass("TRN2", target_bir_lowering=False)

    # All inputs are host-prearranged to exact SBUF layout [128, X] so each
    # loads with ONE dma_start (HWDGE descriptor processing is serial ~625ns
    # per DMA) with maximal contiguous runs.
    # x and the projection weights arrive as dual fp8: hi = fp8(v*SW) plus a
    # compensation term, so each 128-contraction tile is computed by three
    # fp8 DoubleRow chains (hi*x8, lo*x8, hi*xe) at 0.75x the bf16 cost with
    # ~bf16 accuracy (PSUM accumulates in f32; Act rescales by 1/SW).
    xc = [nc.declare_dram_parameter(f"xc{c}", [128, NT * SC], FP8,
                                    isOutput=False) for c in range(NSC)]
    xe = [nc.declare_dram_parameter(f"xe{c}", [128, NT * SC], FP8,
                                    isOutput=False) for c in range(NSC)]
    wq8 = nc.declare_dram_parameter("wq8", [128, NT * EG], FP8, isOutput=False)
    wq8e = nc.declare_dram_parameter("wq8e", [128, NT * EG], FP8, isOutput=False)
    wk8 = nc.declare_dram_parameter("wk8", [128, NT * HEAD_DIM], FP8, isOutput=False)
    wk8e = nc.declare_dram_parameter("wk8e", [128, NT * HEAD_DIM], FP8, isOutput=False)
    wv8 = nc.declare_dram_parameter("wv8", [128, NT * HEAD_DIM], FP8, isOutput=False)
    wv8e = nc.declare_dram_parameter("wv8e", [128, NT * HEAD_DIM], FP8, isOutput=False)
    wo = nc.declare_dram_parameter("wo", [128, GQ * D], BF16, isOutput=False)
    biases = nc.declare_dram_parameter("biases", [128, 6], F32, isOutput=False)
    masks = nc.declare_dram_parameter("masks", [128, 128], BF16, isOutput=False)
    outT = nc.declare_dram_parameter("outT", [D, S], BF16, isOutput=True)

    with tile.TileContext(nc) as tc:
        with (
            tc.tile_pool(name="persist", bufs=1) as pp,
            tc.tile_pool(name="rot", bufs=1) as rp,
            tc.tile_pool(name="ps", bufs=1, space="PSUM") as ps,
        ):
            # ---- constants (no DMA deps) ----
            ones_sq = pp.tile([128, 128], BF16, name="ones_sq")
            nc.vector.memset(ones_sq[:], 1.0)

            # ---- batched loads, priority order (HWDGE is serial) ----
            wk_sb = pp.tile([128, NT * HEAD_DIM], FP8, name="wk_sb")
            nc.sync.dma_start(wk_sb[:], wk8[:])
            wke_sb = pp.tile([128, NT * HEAD_DIM], FP8, name="wke_sb")
            nc.sync.dma_start(wke_sb[:], wk8e[:])
            x_sb = [None] * NSC
            xe_sb = [None] * NSC
            x0_parts = []
            for q in range(2):
                t = pp.tile([128, NT * SC // 2], FP8, name=f"x_sb0{q}")
                nc.sync.dma_start(
                    t[:], xc[0][:, q * NT * SC // 2:(q + 1) * NT * SC // 2])
                x0_parts.append(t)
            xe_sb[0] = pp.tile([128, NT * SC], FP8, name="xe_sb0")
            nc.sync.dma_start(xe_sb[0][:], xe[0][:])
            wv_sb = pp.tile([128, NT * HEAD_DIM], FP8, name="wv_sb")
            nc.sync.dma_start(wv_sb[:], wv8[:])
            wve_sb = pp.tile([128, NT * HEAD_DIM], FP8, name="wve_sb")
            nc.sync.dma_start(wve_sb[:], wv8e[:])
            wq_sb = pp.tile([128, NT * EG], FP8, name="wq_sb")
            nc.sync.dma_start(wq_sb[:], wq8[:])
            wqe_sb = pp.tile([128, NT * EG], FP8, name="wqe_sb")
            nc.sync.dma_start(wqe_sb[:], wq8e[:])
            b_sb = pp.tile([128, 6], F32, name="b_sb")
            nc.sync.dma_start(b_sb[:], biases[:])
            m_sb = pp.tile([128, 128], BF16, name="m_sb")
            nc.sync.dma_start(m_sb[:], masks[:])
            for c in range(1, NSC):
                x_sb[c] = pp.tile([128, NT * SC], FP8, name=f"x_sb{c}")
                nc.sync.dma_start(x_sb[c][:], xc[c][:])
                xe_sb[c] = pp.tile([128, NT * SC], FP8, name=f"xe_sb{c}")
                nc.sync.dma_start(xe_sb[c][:], xe[c][:])
            wo_sb = pp.tile([128, GQ * D], BF16, name="wo_sb")
            nc.sync.dma_start(wo_sb[:], wo[:])

            def x_pair(p, c):
                # moving AP [128, 2, SC] for dtile pair (2p, 2p+1)
                if c == 0:
                    t = x0_parts[p // 4]
                    j = p % 4
                    return t[:].rearrange("q (t n) -> q t n", n=SC)[
                        :, 2 * j:2 * j + 2, :]
                return x_sb[c][:].rearrange("q (t n) -> q t n", n=SC)[
                    :, 2 * p:2 * p + 2, :]

            def xe_pair(p, c):
                return xe_sb[c][:].rearrange("q (t n) -> q t n", n=SC)[
                    :, 2 * p:2 * p + 2, :]

            def wkv_pair(t, p):
                return t[:].rearrange("q (t n) -> q t n", n=HEAD_DIM)[
                    :, 2 * p:2 * p + 2, :]

            def wq_pair(t, p, h):
                return t[:].rearrange("q (t n) -> q t n", n=EG)[
                    :, 2 * p:2 * p + 2, h * 128:(h + 1) * 128]

            def wo_t(et, ft):
                return wo_sb[:, et * D + ft * 128:et * D + (ft + 1) * 128]

            def dr_chains(psum, w_hi, w_lo, xf, xef, c):
                # 3 fp8-DR chains x 8 dtile pairs accumulating x@w in psum:
                # hi*x8 + lo*x8 + hi*xe  (lo, xe carry the 1/32-scaled
                # quantization residuals of w*SW and x)
                chains = ((w_hi, xf), (w_lo, xf), (w_hi, xef))
                n = len(chains) * (NT // 2)
                i = 0
                for wsel, xsel in chains:
                    for p in range(NT // 2):
                        nc.tensor.matmul(
                            psum[:], wsel(p), xsel(p, c),
                            start=(i == 0), stop=(i == n - 1),
                            perf_mode=DRM)
                        i += 1

            KT_sb, VT_sb, V_sb = [None] * NSC, [None] * NSC, [None] * NST
            QT = [[None] * NSC for _ in range(GQ)]
            Ynorm = [[None] * NSC for _ in range(GQ)]

            def projections(c):
                # K/V projections for chunk c (dual-split fp8 DR chains)
                for w_hi_sb, w_lo_sb, bias_ap, out_list, label in (
                    (wk_sb, wke_sb, b_sb[:, 4:5], KT_sb, "KT"),
                    (wv_sb, wve_sb, b_sb[:, 5:6], VT_sb, "VT"),
                ):
                    psum = ps.tile([128, SC], F32, name=f"{label}ps{c}",
                                   tag="mm", bufs=5)
                    dr_chains(psum,
                              lambda p, t=w_hi_sb: wkv_pair(t, p),
                              lambda p, t=w_lo_sb: wkv_pair(t, p),
                              x_pair, xe_pair, c)
                    o = pp.tile([128, SC], BF16, name=f"{label}{c}")
                    nc.scalar.activation(o[:], psum[:], ACT.Identity,
                                         bias=bias_ap, scale=1.0 / SW)
                    out_list[c] = o

                # V natural tiles for this chunk (DMA transpose, off PE)
                for j in range(4):
                    st = 4 * c + j
                    v = pp.tile([128, 128], BF16, name=f"V{st}")
                    nc.sync.dma_start(
                        v[:], VT_sb[c][:, j * 128:(j + 1) * 128],
                        transpose=True)
                    V_sb[st] = v

                # Q projections for all heads, chunk c
                for h in range(GQ):
                    psum = ps.tile([128, SC], F32, name=f"QTps{h}_{c}",
                                   tag="mm", bufs=5)
                    dr_chains(psum,
                              lambda p, h=h: wq_pair(wq_sb, p, h),
                              lambda p, h=h: wq_pair(wqe_sb, p, h),
                              x_pair, xe_pair, c)
                    q = pp.tile([128, SC], BF16, name=f"QT{h}_{c}")
                    nc.scalar.activation(q[:], psum[:], ACT.Identity,
                                         bias=b_sb[:, h:h + 1], scale=1.0 / SW)
                    QT[h][c] = q

            def attention(c, interleave=False):
                n_kt = 4 * (c + 1)
                y_ps_l, sumacc_l = {}, {}

                def attn_step(h, kt, first=None, last=None):
                    if first is None:
                        first = (kt == 0)
                    if last is None:
                        last = (kt == n_kt - 1)
                    if first:
                        y_ps_l[h] = ps.tile([128, SC], F32, name=f"yps{h}_{c}",
                                            tag="y", bufs=2)
                        sumacc_l[h] = rp.tile([128, SC], F32, name=f"sum{h}_{c}",
                                              tag="sumacc", bufs=3)
                    y_ps, sumacc = y_ps_l[h], sumacc_l[h]
                    # diagonal blocks: columns j < 128r are fully masked —
                    # compute only the live subrange [j0:].
                    r = kt - 4 * c
                    j0 = 128 * r if r > 0 else 0
                    s_ps = ps.tile([128, SC], F32, name=f"sps{h}_{c}_{kt}",
                                   tag="mm", bufs=5)
                    nc.tensor.matmul(
                        s_ps[:, j0:],
                        KT_sb[kt // 4][:, (kt % 4) * 128:(kt % 4 + 1) * 128],
                        QT[h][c][:, j0:], start=True, stop=True)
                    a = rp.tile([128, SC], BF16, name=f"A{h}_{c}_{kt}",
                                tag="A", bufs=28)
                    nc.scalar.activation(a[:, j0:], s_ps[:, j0:], ACT.Exp)
                    if r >= 0:
                        # only the 128-wide diagonal block is partially
                        # masked; columns beyond it are fully unmasked
                        nc.vector.tensor_mul(a[:, j0:j0 + 128],
                                             a[:, j0:j0 + 128], m_sb[:])
                    nc.tensor.matmul(y_ps[:, j0:], V_sb[kt][:], a[:, j0:],
                                     start=first, stop=last)
                    if first:
                        nc.vector.tensor_copy(sumacc[:], a[:])
                    else:
                        nc.vector.tensor_add(sumacc[:, j0:], sumacc[:, j0:],
                                             a[:, j0:])

                def attn_tail(h):
                    y_ps, sumacc = y_ps_l[h], sumacc_l[h]
                    sum_bf = rp.tile([128, SC], BF16, name=f"sumbf{h}_{c}",
                                     tag="sumbf", bufs=3)
                    nc.vector.tensor_copy(sum_bf[:], sumacc[:])
                    # ones-matrix matmul = partition sum broadcast to all 128
                    # rows in one shot; reciprocal then runs on all lanes.
                    sum_ps = ps.tile([128, SC], F32, name=f"sumps{h}_{c}",
                                     tag="small", bufs=1)
                    nc.tensor.matmul(sum_ps[:], ones_sq[:], sum_bf[:],
                                     start=True, stop=True)
                    rb_sb = rp.tile([128, SC], F32, name=f"rb{h}_{c}",
                                    bufs=3, tag="rb")
                    nc.vector.reciprocal(rb_sb[:], sum_ps[:])
                    yn = pp.tile([128, SC], BF16, name=f"Yn{h}_{c}")
                    nc.vector.tensor_mul(yn[:], y_ps[:], rb_sb[:])
                    Ynorm[h][c] = yn

                if interleave:
                    for hp in range(0, GQ, 2):
                        for kt in range(n_kt):
                            attn_step(hp, kt)
                            attn_step(hp + 1, kt)
                        attn_tail(hp)
                        attn_tail(hp + 1)
                    return
                for h in range(GQ):
                    for kt in range(n_kt):
                        attn_step(h, kt)
                    attn_tail(h)

            def outproj(c):
                o_stage = rp.tile([128, NT * SC], BF16, name=f"ostage{c}",
                                  tag="ostage", bufs=1)
                for ft in range(NT):
                    o_ps = ps.tile([128, SC], F32, name=f"ops{c}_{ft}",
                                   tag="mm", bufs=5)
                    for et in range(GQ):
                        nc.tensor.matmul(
                            o_ps[:],
                            wo_t(et, ft), Ynorm[et][c][:],
                            start=(et == 0), stop=(et == GQ - 1))
                    dst = o_stage[:, ft * SC:(ft + 1) * SC]
                    if ft % 2 == 0:
                        nc.scalar.activation(dst, o_ps[:], ACT.Copy)
                    else:
                        nc.vector.tensor_copy(dst, o_ps[:])
                outT_v = outT.rearrange("(n p) s -> p n s", p=128)[
                    :, :, c * SC:(c + 1) * SC]
                o_stage_v = o_stage[:].rearrange("p (n j) -> p n j", j=SC)
                nsplit = 8 if c == NSC - 1 else 2
                step = NT // nsplit
                for qq in range(nsplit):
                    nc.gpsimd.dma_start(outT_v[:, qq * step:(qq + 1) * step],
                                        o_stage_v[:, qq * step:(qq + 1) * step])

            projections(0)
            for c in range(NSC):
                attention(c, interleave=(c >= 1))
                if c + 1 < NSC:
                    projections(c + 1)
                outproj(c)

    _split_multiwait(nc)
    return nc


_SPLIT_N = [0]


def _split_multiwait(nc):
    """Rewrite instructions carrying >1 semaphore wait.

    The walrus build here allows one sync wait per instruction; Tile's
    wait-assignment freely attaches several. Hoist all but the last wait
    onto fresh single-wait NoOps inserted just before the instruction in
    its basic block (engine streams are in-order, so semantics are
    unchanged).
    """
    for f in nc.m.functions:
        for bb in f.blocks:
            il = bb.instructions
            if not any(i.sync_info is not None and len(i.sync_info.on_wait) > 1
                       for i in il):
                continue
            new = []
            for inst in il:
                si = inst.sync_info
                if si is not None and len(si.on_wait) > 1:
                    waits = list(si.on_wait)
                    for w in waits[:-1]:
                        _SPLIT_N[0] += 1
                        new.append(mybir.InstNoOp(
                            name=f"I-waitsplit{_SPLIT_N[0]}",
                            engine=inst.engine,
                            bass_nofuse=True,
                            sync_info=mybir.SyncInfo(on_wait=[w], on_update=[]),
                        ))
                    inst.sync_info = mybir.SyncInfo(
                        on_wait=[waits[-1]], on_update=list(si.on_update))
                new.append(inst)
            bb.instructions = new


def _fp8_dual(a):
    """fp8 dual split of a*SW: (hi, lo) with hi + lo ~ a*SW to ~0.1%.
    lo holds the hi-quantization residual at 1/32 scale (re-quantized)."""
    e4 = ml_dtypes.float8_e4m3
    hi = (a * SW).astype(e4)
    lo = ((a * SW - hi.astype(np.float32)) * 32.0).astype(e4)
    lo32 = (lo.astype(np.float32) / 32.0).astype(e4)
    return hi, lo32


def _host_prep(x, wq_w, wq_b, wk_w, wk_b, wv_w, wv_b, wo_w, wo_b):
    """Build the 8 per-core input maps, prearranged to SBUF layout."""
    bf16 = ml_dtypes.bfloat16
    e4 = ml_dtypes.float8_e4m3
    scale = np.float32(1.0 / math.sqrt(HEAD_DIM))

    def to_sbuf_rows(a2d, width):
        # [NT*128, width] -> [128, NT*width] with column blocks = row tiles
        nt = a2d.shape[0] // 128
        return np.ascontiguousarray(
            a2d.reshape(nt, 128, width).transpose(1, 0, 2).reshape(128, nt * width))

    xc_b, xe_b = [], []
    for b in range(B):
        xbT = np.ascontiguousarray(x[b].T)             # [d, s] f32
        x8 = xbT.astype(e4)
        xr = ((xbT - x8.astype(np.float32)) * 32.0).astype(e4)
        xe32 = (xr.astype(np.float32) / 32.0).astype(e4)
        cb, eb = [], []
        for arr, out in ((x8, cb), (xe32, eb)):
            a4 = arr.reshape(NT, 128, NSC, SC)         # [dt, p, c, j]
            for c in range(NSC):
                out.append(np.ascontiguousarray(
                    a4[:, :, c, :].transpose(1, 0, 2).reshape(128, NT * SC)))
        xc_b.append(cb)
        xe_b.append(eb)

    jj = np.arange(128, dtype=np.int32)[None, :]
    pp_ = np.arange(128, dtype=np.int32)[:, None]
    masks = (jj >= pp_).astype(bf16)

    per_g = []
    for g in range(N_KV):
        wqT = np.ascontiguousarray(
            (wq_w[g * EG:(g + 1) * EG, :] * scale).T)      # [d, 512] f32
        wkT = np.ascontiguousarray(
            wk_w[g * HEAD_DIM:(g + 1) * HEAD_DIM, :].T)    # [d, 128]
        wvT = np.ascontiguousarray(
            wv_w[g * HEAD_DIM:(g + 1) * HEAD_DIM, :].T)
        woT = wo_w[:, g * EG:(g + 1) * EG].T.astype(bf16)  # [512, f]
        wq_hi, wq_lo = _fp8_dual(wqT)
        wk_hi, wk_lo = _fp8_dual(wkT)
        wv_hi, wv_lo = _fp8_dual(wvT)
        biases = np.empty((128, 6), np.float32)
        biases[:, :GQ] = (wq_b[g * EG:(g + 1) * EG] * scale).reshape(GQ, 128).T
        biases[:, 4] = wk_b[g * HEAD_DIM:(g + 1) * HEAD_DIM]
        biases[:, 5] = wv_b[g * HEAD_DIM:(g + 1) * HEAD_DIM]
        per_g.append(dict(
            wq8=to_sbuf_rows(wq_hi, EG),
            wq8e=to_sbuf_rows(wq_lo, EG),
            wk8=to_sbuf_rows(wk_hi, HEAD_DIM),
            wk8e=to_sbuf_rows(wk_lo, HEAD_DIM),
            wv8=to_sbuf_rows(wv_hi, HEAD_DIM),
            wv8e=to_sbuf_rows(wv_lo, HEAD_DIM),
            wo=to_sbuf_rows(woT, D),
            biases=biases,
        ))

    in_maps = []
    for core in range(8):
        b, g = divmod(core, N_KV)
        m = dict(per_g[g])
        for c in range(NSC):
            m[f"xc{c}"] = xc_b[b][c]
            m[f"xe{c}"] = xe_b[b][c]
        m["masks"] = masks
        in_maps.append(m)
    return in_maps


def kernel(x, wq_w, wq_b, wk_w, wk_b, wv_w, wv_b, wo_w, wo_b, **run_kwargs):
    x = np.asarray(x, dtype=np.float32)
    wq_w = np.asarray(wq_w, dtype=np.float32)
    wq_b = np.asarray(wq_b, dtype=np.float32)
    wk_w = np.asarray(wk_w, dtype=np.float32)
    wk_b = np.asarray(wk_b, dtype=np.float32)
    wv_w = np.asarray(wv_w, dtype=np.float32)
    wv_b = np.asarray(wv_b, dtype=np.float32)
    wo_w = np.asarray(wo_w, dtype=np.float32)
    wo_b = np.asarray(wo_b, dtype=np.float32)

    if "nc" not in _CACHE:
        _CACHE["nc"] = _build_nc()
    nc = _CACHE["nc"]

    in_maps = _host_prep(x, wq_w, wq_b, wk_w, wk_b, wv_w, wv_b, wo_w, wo_b)
    res = run_bass_kernel_spmd(nc, in_maps, core_ids=list(range(8)),
                               **run_kwargs)

    out = np.empty((B, S, D), dtype=np.float32)
    for b in range(B):
        acc = res.results[b * N_KV]["outT"].astype(np.float32)
        for g in range(1, N_KV):
            acc = acc + res.results[b * N_KV + g]["outT"].astype(np.float32)
        out[b] = acc.T + wo_b[None, :]
    _CACHE["last_res"] = res
    return out

